# revision 1
# baseline (speedup 1.0000x reference)
"""Trainium2 Bass kernel for nn_BeliefDynamics.

Math reduction of the reference:
  - _total_log_prob is quadratic in z, so its Hessian is the constant
    matrix H = -(1/NOISE_SCALE^2 + 1) I.  Hence
       drift_matrix = -H - H^T + 2*DIFF*I = d * I   (scalar d)
       exp_drift    = expm(d*DT * I) = c * I        (scalar c)
       new_cov      = c^2 * covariance
    and the eigh/clip/regularize step is a numerical no-op for this
    well-conditioned SPD input (eigenvalues ~[1, 2.7] * c^2, condition
    number << 1e6, min eigenvalue >> 1e-8).
  - precision = inv(new_cov + 1e-8 I) = (1/c^2) * inv(covariance)
    (the 1e-8 shift is ~2e-10 relative: below fp32 resolution).
  - new_mean is a cheap elementwise vector update.

So the real work is one 1024x1024 SPD inverse.  We compute it with a
degree-3 Chebyshev polynomial initialization followed by two
Newton-Schulz steps, column-block-sharded over 8 NeuronCores:

  per core j (owning 128 columns):
    M2 = Mbf^T Mbf_j          (bf16)
    M3 = Mbf^T (M2hi+M2lo)_j  (bf16, hi/lo split keeps it a polynomial
                               in the SAME matrix Mbf -> commutators
                               cancel, so X1 stays nearly symmetric)
    X1 = c0 I + c1 M + c2 M2 + c3 M3          (Chebyshev approx of 1/x)
    AllGather(X1, bf16)       [overlaps the next fp32 product]
    R1 = I - M^T X1_j         (true fp32 product: residual measurement)
    X2 = X1 + X1full^T R1_j   (bf16 correction product)
    AllGather(X2, bf16)
    R2 = I - M^T X2_j         (fp32)
    prec_j = (X2 + X2full^T R2_j) / c^2
    ncov_j = c^2 * M_j

All heavy compute runs on the 8 NeuronCores; the host only computes
scalar constants (spectral-interval estimate via power iteration ->
Chebyshev coefficients, and the expm scalar) and slices/reassembles
blocks.
"""

import numpy as np
import ml_dtypes

import concourse.bass as bass
import concourse.mybir as mybir
import concourse.tile as tile
from concourse import bacc, bass_utils
from concourse.bass import ts

F32 = mybir.dt.float32
BF16 = mybir.dt.bfloat16
AF = mybir.ActivationFunctionType
OP = mybir.AluOpType

N_CORES = 8
P = 128
D = 1024
KT = D // P  # 8 k-tiles

# ----------------------------------------------------------------------------
# Host-side scalar constants
# ----------------------------------------------------------------------------

DT_ = 0.01
DIFF = 0.1
LR = 0.1
NOISE_SCALE = 0.1


def _expm_scalar():
    """The f32 scalar c with expm(drift_matrix*DT) == c*I, mirroring the
    reference's jax computation (expm of a*I is exactly r(a)*I where r is
    the same scalar Pade evaluation as on a 1x1 matrix)."""
    import jax
    import jax.numpy as jnp
    from jax.scipy.linalg import expm

    # Hessian of the quadratic _total_log_prob (constant, diagonal).
    def tlp(z, obs, score):
        obs_lp = -0.5 * jnp.sum((z - obs) ** 2) / (NOISE_SCALE**2)
        prior_lp = -0.5 * jnp.sum(z**2)
        return obs_lp + prior_lp + jnp.sum(z * score)

    cpu = jax.devices("cpu")[0]
    with jax.default_device(cpu):
        z = jnp.zeros((2,), jnp.float32)
        H = jax.hessian(tlp)(z, z, z)
        H = 0.5 * (H + H.T)
        h00 = np.float32(np.asarray(H)[0, 0])
        # drift = -H - H^T + 2*DIFF*I  (diagonal value, f32 ops)
        dval = np.float32(np.float32(-h00) - h00) + np.float32(2.0 * DIFF)
        a = np.float32(dval * np.float32(DT_))
        c = np.asarray(expm(jnp.asarray([[a]], jnp.float32)))[0, 0]
    return np.float32(c)


def _lam_bounds(S):
    """Cheap spectral-interval estimate for the SPD matrix S (power
    iteration for lambda_max, shifted power iteration for lambda_min)."""
    rng = np.random.default_rng(12345)
    v = rng.standard_normal(D).astype(np.float32)
    v /= np.linalg.norm(v)
    lmax = 1.0
    for _ in range(40):
        w = S @ v
        lmax = float(v @ w)
        nw = np.linalg.norm(w)
        if not np.isfinite(nw) or nw == 0:
            return 0.5, 4.0
        v = w / nw
    shift = np.float32(lmax * 1.05 + 0.1)
    v = rng.standard_normal(D).astype(np.float32)
    v /= np.linalg.norm(v)
    for _ in range(60):
        w = shift * v - S @ v
        mu = float(v @ w)
        nw = np.linalg.norm(w)
        if not np.isfinite(nw) or nw == 0:
            return 0.5, 4.0
        v = w / nw
    lmin = shift - mu
    return lmin, lmax


def _cheb_inv_coeffs(a, b):
    """Power-basis coefficients of the degree-3 Chebyshev interpolant of
    1/x on [a, b] (near-minimax)."""
    ch = np.polynomial.Chebyshev.interpolate(lambda x: 1.0 / x, 3, domain=[a, b])
    p = ch.convert(kind=np.polynomial.Polynomial)
    c = np.zeros(4)
    c[: len(p.coef)] = p.coef
    return [float(x) for x in c]


# ----------------------------------------------------------------------------
# Device kernel
# ----------------------------------------------------------------------------

_BUILD_CACHE = {}


def _build(key):
    (c0, c1, c2, c3, s2, inv_s2) = key
    nc = bacc.Bacc("TRN2", target_bir_lowering=False, debug=False,
                   num_devices=N_CORES)

    # --- I/O ---
    mfull = nc.dram_tensor("mfull", [D, D], F32, kind="ExternalInput")
    mbf = nc.dram_tensor("mbf", [D, D], BF16, kind="ExternalInput")
    mblk = nc.dram_tensor("mblk", [P, D], F32, kind="ExternalInput")
    mblkbf = nc.dram_tensor("mblkbf", [P, D], BF16, kind="ExternalInput")
    eyeb = nc.dram_tensor("eyeb", [P, D], F32, kind="ExternalInput")
    vmean = nc.dram_tensor("vmean", [D], F32, kind="ExternalInput")
    vobs = nc.dram_tensor("vobs", [D], F32, kind="ExternalInput")
    vscore = nc.dram_tensor("vscore", [D], F32, kind="ExternalInput")
    vnoise = nc.dram_tensor("vnoise", [D], F32, kind="ExternalInput")

    prec_o = nc.dram_tensor("prec", [P, D], F32, kind="ExternalOutput")
    ncov_o = nc.dram_tensor("ncov", [P, D], F32, kind="ExternalOutput")
    nmean_o = nc.dram_tensor("nmean", [D], F32, kind="ExternalOutput")

    m100 = float(np.float32(1.0) / np.float32(NOISE_SCALE**2))
    cn = float(np.float32(np.sqrt(np.float32(2.0 * DIFF * DT_))) *
               np.float32(NOISE_SCALE))

    with tile.TileContext(nc) as tc:
        with (
            tc.tile_pool(name="const", bufs=1) as const,
            tc.tile_pool(name="work", bufs=1) as work,
            tc.tile_pool(name="scr", bufs=3) as scr,
            tc.tile_pool(name="gat", bufs=2) as gat,
            tc.tile_pool(name="pp", bufs=4, space="PSUM") as ppool,
            tc.tile_pool(name="psn", bufs=2, space="PSUM") as psn,
            tc.tile_pool(name="dram", bufs=1, space="DRAM") as dpool,
        ):
            # ---------------- loads ----------------
            mf_sb = const.tile([P, KT, D], F32)
            nc.sync.dma_start(mf_sb[:], mfull.ap().rearrange("(t p) q -> p t q", p=P))
            mbf_sb = const.tile([P, KT, D], BF16)
            nc.sync.dma_start(mbf_sb[:], mbf.ap().rearrange("(t p) q -> p t q", p=P))
            mblk_sb = const.tile([P, D], F32)
            nc.sync.dma_start(mblk_sb[:], mblk.ap())
            mblkbf_sb = const.tile([P, D], BF16)
            nc.sync.dma_start(mblkbf_sb[:], mblkbf.ap())
            eye_sb = const.tile([P, D], F32)
            nc.sync.dma_start(eye_sb[:], eyeb.ap())

            vm_sb = const.tile([P, D // P], F32)
            nc.sync.dma_start(vm_sb[:], vmean.ap().rearrange("(p f) -> p f", p=P))
            vo_sb = const.tile([P, D // P], F32)
            nc.sync.dma_start(vo_sb[:], vobs.ap().rearrange("(p f) -> p f", p=P))
            vs_sb = const.tile([P, D // P], F32)
            nc.sync.dma_start(vs_sb[:], vscore.ap().rearrange("(p f) -> p f", p=P))
            vn_sb = const.tile([P, D // P], F32)
            nc.sync.dma_start(vn_sb[:], vnoise.ap().rearrange("(p f) -> p f", p=P))

            # ---------------- mean path ----------------
            NF = D // P
            g = work.tile([P, NF], F32)
            nc.vector.tensor_tensor(g[:], vm_sb[:], vo_sb[:], OP.subtract)
            nc.vector.tensor_scalar_mul(g[:], g[:], -m100)
            nc.vector.tensor_tensor(g[:], g[:], vm_sb[:], OP.subtract)
            nc.vector.tensor_tensor(g[:], g[:], vs_sb[:], OP.add)
            gsq = work.tile([P, NF], F32)
            nc.vector.tensor_tensor(gsq[:], g[:], g[:], OP.mult)
            gr = work.tile([P, 1], F32)
            nc.vector.reduce_sum(gr[:], gsq[:], axis=mybir.AxisListType.X)
            ones = const.tile([P, 1], F32)
            nc.vector.memset(ones[:], 1.0)
            nsq = psn.tile([1, 1], F32)
            nc.tensor.matmul(nsq[:], gr[:], ones[:], start=True, stop=True)
            gnorm = work.tile([1, 1], F32)
            nc.scalar.activation(gnorm[:], nsq[:], AF.Sqrt)
            denom = work.tile([1, 1], F32)
            nc.vector.tensor_scalar(denom[:], gnorm[:], 0.1, 1.0, OP.mult, OP.add)
            adt = work.tile([1, 1], F32)
            nc.vector.reciprocal(adt[:], denom[:])
            nc.vector.tensor_scalar_mul(adt[:], adt[:], float(np.float32(DT_)))
            adtb = work.tile([P, 1], F32)
            nc.gpsimd.partition_broadcast(adtb[:], adt[:1, :])
            drift = work.tile([P, NF], F32)
            nc.vector.tensor_scalar_mul(drift[:], g[:], float(np.float32(-LR)))
            nc.vector.tensor_scalar(drift[:], drift[:], adtb[:, 0:1], None, OP.mult)
            nz = work.tile([P, NF], F32)
            nc.vector.tensor_scalar_mul(nz[:], vn_sb[:], cn)
            nm = work.tile([P, NF], F32)
            nc.vector.tensor_tensor(nm[:], vm_sb[:], drift[:], OP.add)
            nc.vector.tensor_tensor(nm[:], nm[:], nz[:], OP.add)
            nc.sync.dma_start(nmean_o.ap().rearrange("(p f) -> p f", p=P), nm[:])

            # ---------------- new_cov (independent, early) ----------------
            ncov_sb = work.tile([P, D], F32)
            nc.any.tensor_scalar_mul(ncov_sb[:], mblk_sb[:], s2)
            nc.sync.dma_start(ncov_o.ap(), ncov_sb[:])

            # ---------------- P1: M2 = Mbf^T Mbf_j ----------------
            x1 = work.tile([P, D], F32)
            m2hi = work.tile([P, D], BF16)
            m2lo = work.tile([P, D], BF16)
            for m in range(KT):
                pp = ppool.tile([P, P], F32, tag="pp", name="pp")
                for k in range(KT):
                    nc.tensor.matmul(pp[:], mbf_sb[:, k, ts(m, P)],
                                     mblkbf_sb[:, ts(k, P)],
                                     start=(k == 0), stop=(k == KT - 1))
                nc.any.tensor_scalar_mul(x1[:, ts(m, P)], pp[:], c2)
                nc.any.tensor_copy(m2hi[:, ts(m, P)], pp[:])
                nc.vector.tensor_tensor(m2lo[:, ts(m, P)], pp[:],
                                        m2hi[:, ts(m, P)], OP.subtract)

            # ---------------- P2: M3 = Mbf^T (M2hi + M2lo)_j ----------------
            for m in range(KT):
                pp = ppool.tile([P, P], F32, tag="pp", name="pp")
                i = 0
                for k in range(KT):
                    for rb in (m2hi, m2lo):
                        nc.tensor.matmul(pp[:], mbf_sb[:, k, ts(m, P)],
                                         rb[:, ts(k, P)],
                                         start=(i == 0), stop=(i == 2 * KT - 1))
                        i += 1
                t3 = scr.tile([P, P], F32, tag="t3", name="t3")
                nc.any.tensor_scalar_mul(t3[:], pp[:], c3)
                nc.vector.tensor_tensor(x1[:, ts(m, P)], x1[:, ts(m, P)],
                                        t3[:], OP.add)

            # X1 += c1*M_j + c0*I_j ; cast to bf16; AllGather
            t1 = work.tile([P, D], F32)
            nc.any.tensor_scalar_mul(t1[:], mblk_sb[:], c1)
            nc.vector.tensor_tensor(x1[:], x1[:], t1[:], OP.add)
            t2 = work.tile([P, D], F32)
            nc.any.tensor_scalar_mul(t2[:], eye_sb[:], c0)
            nc.vector.tensor_tensor(x1[:], x1[:], t2[:], OP.add)
            x1bf = work.tile([P, D], BF16)
            nc.any.tensor_copy(x1bf[:], x1[:])

            b1 = dpool.tile([P, D], BF16)
            g1 = dpool.tile([D, D], BF16, addr_space="Shared")
            nc.sync.dma_start(b1[:], x1bf[:])
            nc.gpsimd.collective_compute(
                "AllGather", OP.bypass,
                replica_groups=[list(range(N_CORES))],
                ins=[b1[:].opt()], outs=[g1[:].opt()])
            x1full = gat.tile([P, KT, D], BF16, tag="xfull", name="x1full")
            nc.sync.dma_start(x1full[:], g1[:].rearrange("(t p) q -> p t q", p=P))

            # ---------------- P3: R1 = I - M^T X1_j  (fp32) ----------------
            r1bf = work.tile([P, D], BF16)
            for m in range(KT):
                pp = ppool.tile([P, P], F32, tag="pp", name="pp")
                for k in range(KT):
                    nc.tensor.matmul(pp[:], mf_sb[:, k, ts(m, P)],
                                     x1[:, ts(k, P)],
                                     start=(k == 0), stop=(k == KT - 1))
                nc.vector.tensor_tensor(r1bf[:, ts(m, P)], eye_sb[:, ts(m, P)],
                                        pp[:], OP.subtract)

            # ---------------- P4: X2 = X1 + X1full^T R1_j (bf16) ----------
            x2 = work.tile([P, D], F32)
            for m in range(KT):
                pp = ppool.tile([P, P], F32, tag="pp", name="pp")
                for k in range(KT):
                    nc.tensor.matmul(pp[:], x1full[:, m, ts(k, P)],
                                     r1bf[:, ts(k, P)],
                                     start=(k == 0), stop=(k == KT - 1))
                nc.vector.tensor_tensor(x2[:, ts(m, P)], x1[:, ts(m, P)],
                                        pp[:], OP.add)
            x2bf = work.tile([P, D], BF16)
            nc.any.tensor_copy(x2bf[:], x2[:])

            b2 = dpool.tile([P, D], BF16)
            g2 = dpool.tile([D, D], BF16, addr_space="Shared")
            nc.sync.dma_start(b2[:], x2bf[:])
            nc.gpsimd.collective_compute(
                "AllGather", OP.bypass,
                replica_groups=[list(range(N_CORES))],
                ins=[b2[:].opt()], outs=[g2[:].opt()])
            x2full = gat.tile([P, KT, D], BF16, tag="xfull", name="x2full")
            nc.sync.dma_start(x2full[:], g2[:].rearrange("(t p) q -> p t q", p=P))

            # ---------------- P5: R2 = I - M^T X2_j  (fp32) ----------------
            r2bf = work.tile([P, D], BF16)
            for m in range(KT):
                pp = ppool.tile([P, P], F32, tag="pp", name="pp")
                for k in range(KT):
                    nc.tensor.matmul(pp[:], mf_sb[:, k, ts(m, P)],
                                     x2[:, ts(k, P)],
                                     start=(k == 0), stop=(k == KT - 1))
                nc.vector.tensor_tensor(r2bf[:, ts(m, P)], eye_sb[:, ts(m, P)],
                                        pp[:], OP.subtract)

            # ---------------- P6: prec = (X2 + X2full^T R2_j)/c^2 ----------
            prec_sb = work.tile([P, D], F32)
            for m in range(KT):
                pp = ppool.tile([P, P], F32, tag="pp", name="pp")
                for k in range(KT):
                    nc.tensor.matmul(pp[:], x2full[:, m, ts(k, P)],
                                     r2bf[:, ts(k, P)],
                                     start=(k == 0), stop=(k == KT - 1))
                nc.vector.tensor_tensor(prec_sb[:, ts(m, P)], x2[:, ts(m, P)],
                                        pp[:], OP.add)
            nc.any.tensor_scalar_mul(prec_sb[:], prec_sb[:], inv_s2)
            nc.sync.dma_start(prec_o.ap(), prec_sb[:])

    nc.compile()
    return nc


def _get_nc(key):
    if key not in _BUILD_CACHE:
        _BUILD_CACHE[key] = _build(key)
    return _BUILD_CACHE[key]


# ----------------------------------------------------------------------------
# Host orchestration
# ----------------------------------------------------------------------------

def _prepare(mean, covariance, observation, score_function, noise):
    mean = np.ascontiguousarray(mean, dtype=np.float32)
    cov = np.ascontiguousarray(covariance, dtype=np.float32)
    observation = np.ascontiguousarray(observation, dtype=np.float32)
    score_function = np.ascontiguousarray(score_function, dtype=np.float32)
    noise = np.ascontiguousarray(noise, dtype=np.float32)

    c = _expm_scalar()
    s2 = np.float32(c * c)
    inv_s2 = np.float32(1.0) / s2

    lmin, lmax = _lam_bounds(cov)
    # conservative margins, rounded to a coarse grid for NEFF caching
    a = max(lmin * 0.97 - 1e-3, 1e-6 * lmax)
    b = lmax * 1.03 + 1e-3
    a = max(np.floor(a * 16.0) / 16.0, 1.0 / 1024.0)
    b = np.ceil(b * 16.0) / 16.0
    c0, c1, c2, c3 = _cheb_inv_coeffs(a, b)

    key = (c0, c1, c2, c3, float(s2), float(inv_s2))

    covbf = cov.astype(ml_dtypes.bfloat16)
    eye = np.eye(P, dtype=np.float32)

    in_maps = []
    for j in range(N_CORES):
        blk = cov[:, j * P:(j + 1) * P]          # [1024, 128]
        # SBUF layout [p, kt*128+c] = blk[kt*128+p, c]
        blk_sb = np.ascontiguousarray(
            blk.reshape(KT, P, P).transpose(1, 0, 2).reshape(P, D))
        blkbf_sb = blk_sb.astype(ml_dtypes.bfloat16)
        eyeb = np.zeros((P, D), dtype=np.float32)
        eyeb[:, j * P:(j + 1) * P] = eye
        in_maps.append({
            "mfull": cov,
            "mbf": covbf,
            "mblk": blk_sb,
            "mblkbf": blkbf_sb,
            "eyeb": eyeb,
            "vmean": mean,
            "vobs": observation,
            "vscore": score_function,
            "vnoise": noise,
        })
    return key, in_maps


def _assemble(results):
    new_mean = results[0]["nmean"].copy()
    new_cov = np.empty((D, D), dtype=np.float32)
    precision = np.empty((D, D), dtype=np.float32)
    for j in range(N_CORES):
        for name, dst in (("ncov", new_cov), ("prec", precision)):
            blk_sb = results[j][name]  # [128, 1024] in [p, kt*128+c] layout
            blk = blk_sb.reshape(P, KT, P).transpose(1, 0, 2).reshape(D, P)
            dst[:, j * P:(j + 1) * P] = blk
    return new_mean, new_cov, precision


def run_spmd(mean, covariance, observation, score_function, noise, **kwargs):
    key, in_maps = _prepare(mean, covariance, observation, score_function,
                            noise)
    nc = _get_nc(key)
    res = bass_utils.run_bass_kernel_spmd(
        nc, in_maps, core_ids=list(range(N_CORES)), **kwargs)
    return _assemble(res.results), res


def kernel(mean, covariance, observation, score_function, noise):
    (out, _res) = run_spmd(mean, covariance, observation, score_function,
                           noise)
    return out
